# revision 25
# baseline (speedup 1.0000x reference)
"""BiPairwiseNegativeCELoss Trainium2 kernel (8-core data-parallel).

loss = ( mean(softplus(neg - pos)) + mean(softplus(neg_ib - pos)) ) / 2
  pos    = rowwise dot(q, d)                (diag of q @ d.T)
  neg    = rowwise dot(q, nd)
  neg_ib = rowmax of (q @ d.T - BIG*eye)    (hardest in-batch negative)

Sharding: batch rows split across 8 cores (2048 rows each); every core
streams the full doc matrix as the matmul moving operand.

The row-max over 16384 columns is the bottleneck (PSUM can only be read
by the Vector/Scalar engines at ~1 elem/lane/cycle). We halve it with a
pair-max decomposition:

  max(a, b) = (a+b)/2 + |(a-b)/2|

  ssum = q @ DsumT  (Dsum = (d_even + d_odd)/2)      [TensorE]
  sdif = q @ DdifT  (Ddif = (d_even - d_odd)/2)      [TensorE]
  |sdif|           PSUM -> SBUF                       [ScalarE Abs]
  rowmax(ssum + |sdif|), seeded/chained per chunk     [VectorE, one
      fused custom-DVE op: body=Src0+Src1, accum=maxx, seed=C1]

The fused op is registered at import time into concourse's custom-DVE
table mechanism (uop table ships inside the NEFF).

The diagonal pair {s_ii, s_i,i^1} is excluded with a -1e6 "half-eye"
mask on the ssum bank; the partner score s_i,i^1 is re-added exactly as
the chunk-0 seed (computed as a rowwise dot). Per-core pair-columns are
rotated so every core's diagonal block lands in chunk 0 at the same
static position (keeps the program SPMD).
Softplus + means run on the host in float64 on the tiny per-row vectors.
"""

import numpy as np
import ml_dtypes

import concourse.bacc as bacc
import concourse.tile as tile
import concourse.mybir as mybir
import concourse.dve_ops as dve_ops
from concourse.dve_spec import Spec, Src0, Src1, C1, maxx, lower, _has_src1
from concourse.dve_uop import DveOpSpec
from concourse.bass_utils import run_bass_kernel_spmd
from contextlib import ExitStack

B = 16384          # batch
D = 128            # embedding dim
NCORES = 8
R = B // NCORES    # rows per core = 2048
M_TILES = R // 128          # 16 row tiles per core
PC = B // 2                 # pair columns = 8192
CHUNK = 1024                # pair columns per pipeline iteration
N_CHUNKS = PC // CHUNK      # 8
MM_N = 512                  # moving free dim per matmul
BIG = 1e6

_COMPILED = None


def _ref_tt_add_maxred(in0, in1, c0, c1, c2):
    P = in0.shape[0]
    body = (in0.astype(np.float32).reshape(P, -1)
            + np.asarray(in1, np.float32).reshape(P, -1))
    return body, dve_ops._accum_ref(body, c1, maxx, False)


def _register_fused_op():
    """out = in0 + in1 ; accum_out = max(rowmax(out), seed[C1])."""
    name = "TT_ADD_MAXREDUCE_ANT"
    if name in dve_ops._SUB_OPCODE_FOR_NAME:
        return next(op for op in dve_ops.OPS if op.name == name)
    op = dve_ops.DveOp(
        name,
        Spec(body=Src0 + Src1, accum=maxx, accum_init=C1,
             reference=_ref_tt_add_maxred),
        subdim=False,
        uops_sha={},
    )
    row = max(dve_ops._SUB_OPCODE_FOR_NAME.values()) + 1
    assert row < 0x20
    dve_ops.OPS.append(op)
    dve_ops.CUSTOM_DVE_SPECS[name] = op.spec
    dve_ops._SUB_OPCODE_FOR_NAME[name] = row
    for ver in ("v3", "v4"):
        spec = DveOpSpec(name=name, opcode=row, uops=lower(op.spec, ver=ver),
                         rd1_en=_has_src1(op.spec))
        op.uops_sha[ver] = spec.sha(ver)
    return op


FUSED_OP = _register_fused_op()


def _build(repeat=1, absd_f16=True, prefetch_dif=False, no_act=False, no_dve=False):
    fp32, bf16 = mybir.dt.float32, mybir.dt.bfloat16
    absd_dt = mybir.dt.float16 if absd_f16 else fp32
    nc = bacc.Bacc("TRN2", target_bir_lowering=False, debug=False)

    qT_d = nc.dram_tensor("qT", [D, R], bf16, kind="ExternalInput")
    dsumT_d = nc.dram_tensor("dsumT", [D, PC], bf16, kind="ExternalInput")
    ddifT_d = nc.dram_tensor("ddifT", [D, PC], bf16, kind="ExternalInput")
    # (q±x) shards for the rowwise dots via the square trick:
    # 4*q.x = sum((q+x)^2) - sum((q-x)^2), accumulated on ScalarE
    dot_names = ["qd_s", "qd_d", "qn_s", "qn_d", "qw_s", "qw_d"]
    dot_drams = {n: nc.dram_tensor(n, [R, D], fp32, kind="ExternalInput")
                 for n in dot_names}
    heye_d = nc.dram_tensor("heye", [D, 64], fp32, kind="ExternalInput")
    out_d = nc.dram_tensor("out", [5, D, M_TILES], fp32, kind="ExternalOutput")

    with tile.TileContext(nc) as tc, ExitStack() as ctx:
        resid = ctx.enter_context(tc.tile_pool(name="resid", bufs=1))
        dots_in = ctx.enter_context(tc.tile_pool(name="dots_in", bufs=3))
        absp = ctx.enter_context(tc.tile_pool(name="absp", bufs=4))
        small = ctx.enter_context(tc.tile_pool(name="small", bufs=1))
        trashp = ctx.enter_context(tc.tile_pool(name="trashp", bufs=2))
        psum_dif = ctx.enter_context(tc.tile_pool(name="psum_dif", bufs=2, space="PSUM"))
        psum_sum = ctx.enter_context(tc.tile_pool(name="psum_sum", bufs=2, space="PSUM"))

        # resident operands
        qT = resid.tile([D, R], bf16, name="qT_t")
        dsumT = resid.tile([D, PC], bf16, name="dsumT_t")
        ddifT = resid.tile([D, PC], bf16, name="ddifT_t")
        heye = resid.tile([D, 64], fp32, name="heye_t")

        nc.sync.dma_start(qT[:], qT_d.ap())
        nc.sync.dma_start(heye[:], heye_d.ap())
        for ci in range(N_CHUNKS):
            sl = slice(ci * CHUNK, (ci + 1) * CHUNK)
            nc.sync.dma_start(ddifT[:, sl], ddifT_d.ap()[:, sl])
            nc.sync.dma_start(dsumT[:, sl], dsumT_d.ap()[:, sl])

        # staging for per-row results
        accs = {n: small.tile([D, M_TILES], fp32, name=f"acc_{n}")
                for n in dot_names}
        par_acc = small.tile([D, M_TILES], fp32, name="par_acc")
        # chain[ci] holds the running rowmax after chunk ci (per m-tile col)
        chain = [small.tile([D, M_TILES], fp32, name=f"chain_{ci}")
                 for ci in range(N_CHUNKS)]

        # ---- rowwise dots via ScalarE Square+accumulate, early ----
        f16 = mybir.dt.float16
        for m in range(M_TILES):
            rs = slice(m * 128, (m + 1) * 128)
            for n in dot_names:
                xt = dots_in.tile([128, D], fp32, name=f"dot_{n}")
                nc.sync.dma_start(xt[:], dot_drams[n].ap()[rs, :])
                tr = trashp.tile([128, D], f16, name="dot_trash")
                nc.scalar.activation(tr[:], xt[:],
                                     mybir.ActivationFunctionType.Square,
                                     accum_out=accs[n][:, m : m + 1])
        # partner seed: host pre-scales (q±w) by 1/2, so the squared-sum
        # difference is exactly q.w
        nc.vector.tensor_tensor(par_acc[:], accs["qw_s"][:], accs["qw_d"][:],
                                op=mybir.AluOpType.subtract)

        # ---- pair-max pipeline ----
        loop_cm = ExitStack()
        if repeat > 1:
            loop_cm.enter_context(tc.For_i(
                0, repeat, 1,
                hint_engines=(mybir.EngineType.PE, mybir.EngineType.DVE,
                              mybir.EngineType.Activation)))
        iters = [(ci, m) for ci in range(N_CHUNKS) for m in range(M_TILES)]

        def emit_dif(ci, m):
            dif = psum_dif.tile([128, CHUNK], fp32, name="dif_bank")
            w = qT[:, m * 128 : (m + 1) * 128]
            for h in range(CHUNK // MM_N):
                cs = slice(ci * CHUNK + h * MM_N, ci * CHUNK + (h + 1) * MM_N)
                nc.tensor.matmul(dif[:, h * MM_N : (h + 1) * MM_N], w,
                                 ddifT[:, cs], start=True, stop=True)
            return dif

        absd_static = resid.tile([128, CHUNK], absd_dt, name="absd_static")
        if no_act:
            nc.vector.memset(absd_static[:], 0.25)
        if no_dve:
            for ci in range(N_CHUNKS):
                nc.vector.memset(chain[ci][:], 0.0)

        difs = {}
        if prefetch_dif:
            difs[iters[0]] = emit_dif(*iters[0])
        for idx, (ci, m) in enumerate(iters):
            dif = difs.pop((ci, m)) if prefetch_dif else emit_dif(ci, m)
            if no_act:
                absd = absd_static
            else:
                absd = absp.tile([128, CHUNK], absd_dt, name="absd")
                nc.scalar.activation(absd[:], dif[:], mybir.ActivationFunctionType.Abs)

            # optionally prefetch next iteration's dif matmuls
            if prefetch_dif and idx + 1 < len(iters):
                difs[iters[idx + 1]] = emit_dif(*iters[idx + 1])

            sm = psum_sum.tile([128, CHUNK], fp32, name="sum_bank")
            w = qT[:, m * 128 : (m + 1) * 128]
            for h in range(CHUNK // MM_N):
                hs = slice(h * MM_N, (h + 1) * MM_N)
                cs = slice(ci * CHUNK + h * MM_N, ci * CHUNK + (h + 1) * MM_N)
                nc.tensor.matmul(sm[:, hs], w, dsumT[:, cs], start=True, stop=True)
            if no_dve:
                continue
            if ci == 0:
                # mask the diagonal pair block (rotated into chunk 0)
                ms = slice(m * 64, m * 64 + 64)
                nc.vector.tensor_tensor(sm[:, ms], sm[:, ms], heye[:, 0:64],
                                        op=mybir.AluOpType.subtract)
            seed = -1e30 if ci == 0 else chain[ci - 1][:, m : m + 1]
            tr2 = trashp.tile([128, CHUNK], fp32, name="fused_trash")
            nc.vector._custom_dve(
                FUSED_OP, out=tr2[:], in0=sm[:], in1=absd[:],
                s1=seed,
                accum_out=chain[ci][:, m : m + 1])

        # fold the exact partner score back in (replaces the masked diag pair)
        negib = small.tile([D, M_TILES], fp32, name="negib_t")
        if not no_dve:
            nc.vector.tensor_tensor(negib[:], chain[N_CHUNKS - 1][:], par_acc[:],
                                    op=mybir.AluOpType.max)
        else:
            nc.vector.memset(negib[:], 0.0)

        loop_cm.close()

        nc.sync.dma_start(out_d.ap()[0], negib[:])
        nc.sync.dma_start(out_d.ap()[1], accs["qd_s"][:])
        nc.sync.dma_start(out_d.ap()[2], accs["qd_d"][:])
        nc.sync.dma_start(out_d.ap()[3], accs["qn_s"][:])
        nc.sync.dma_start(out_d.ap()[4], accs["qn_d"][:])

    nc.compile()
    return nc


def _get_compiled():
    global _COMPILED
    if _COMPILED is None:
        _COMPILED = _build()
    return _COMPILED


def _prep_inputs(q, d, nd):
    q = np.ascontiguousarray(np.asarray(q, dtype=np.float32))
    d = np.ascontiguousarray(np.asarray(d, dtype=np.float32))
    nd = np.ascontiguousarray(np.asarray(nd, dtype=np.float32))

    qT_bf = np.ascontiguousarray(q.T.astype(ml_dtypes.bfloat16))          # [D, B]
    dsum = ((d[0::2] + d[1::2]) * np.float32(0.5))                         # [PC, D]
    ddif = ((d[0::2] - d[1::2]) * np.float32(0.5))
    dsumT = np.ascontiguousarray(dsum.T.astype(ml_dtypes.bfloat16))        # [D, PC]
    ddifT = np.ascontiguousarray(ddif.T.astype(ml_dtypes.bfloat16))
    dsw = d[np.arange(B) ^ 1]                                              # partner rows

    heye = np.zeros((D, 64), dtype=np.float32)
    heye[np.arange(D), np.arange(D) // 2] = BIG

    half = np.float32(0.5)
    dots_full = {
        "qd_s": (q + d) * half, "qd_d": (q - d) * half,
        "qn_s": (q + nd) * half, "qn_d": (q - nd) * half,
        "qw_s": (q + dsw) * half, "qw_d": (q - dsw) * half,
    }

    in_maps = []
    for c in range(NCORES):
        r0 = c * R
        rot = np.roll(np.arange(PC), -(r0 // 2))
        im = {
            "qT": np.ascontiguousarray(qT_bf[:, r0 : r0 + R]),
            "dsumT": np.ascontiguousarray(dsumT[:, rot]),
            "ddifT": np.ascontiguousarray(ddifT[:, rot]),
            "heye": heye,
        }
        for n, arr in dots_full.items():
            im[n] = np.ascontiguousarray(arr[r0 : r0 + R])
        in_maps.append(im)
    return in_maps


def _gather(results):
    negib = np.empty(B, dtype=np.float32)
    pos = np.empty(B, dtype=np.float32)
    neg = np.empty(B, dtype=np.float32)
    for c in range(NCORES):
        o = results[c]["out"]  # [5, 128, M_TILES]
        r0 = c * R
        negib[r0 : r0 + R] = o[0].T.reshape(-1)
        # dot = sum((q+x)^2)/4 - sum((q-x)^2)/4 with the 1/2 prescale
        # already applied on the host: dot = sum(s^2) - sum(d^2)
        pos[r0 : r0 + R] = (o[1] - o[2]).T.reshape(-1)
        neg[r0 : r0 + R] = (o[3] - o[4]).T.reshape(-1)
    return negib, pos, neg


def kernel(query_embeddings, doc_embeddings, neg_doc_embeddings):
    nc = _get_compiled()
    in_maps = _prep_inputs(query_embeddings, doc_embeddings, neg_doc_embeddings)
    res = run_bass_kernel_spmd(nc, in_maps, core_ids=list(range(NCORES)))
    negib, pos, neg = _gather(res.results)

    pos64 = pos.astype(np.float64)
    l1 = np.mean(np.logaddexp(0.0, neg.astype(np.float64) - pos64))
    l2 = np.mean(np.logaddexp(0.0, negib.astype(np.float64) - pos64))
    return np.float32((l1 + l2) / 2.0)
